# revision 10
# baseline (speedup 1.0000x reference)
"""Trainium2 Bass kernel for batched multi-head attention (B=8, T=2048, C=1024, H=16).

Sharding: data-parallel over batch — one batch element per NeuronCore (8 cores).

Per-core algorithm (all matmul inputs bf16, accumulation/stats f32):
  qT = (Wq^T x^T + bq)        stored [C_q, T]   (head h = partitions h*64..h*64+63 of chunk h//2)
  kT = (Wk^T xc^T + bk)       stored [C_k, TK]  (xc = column-compacted x, see below)
  v  = (xc W_v + bv)          stored [TK, C_v] with a per-head appended column = valid(ki)
                              and every row scaled by valid(ki)  (0 for masked/pad slots)
  S^T[ki,qi] = sum_d kT[d,ki] qT[d,qi]          (PE, K=64)
  P^T = exp(0.125 * S^T)                        (ACT, no max-subtraction: scores are O(1))
  outT[dd,qi] (+ l[qi] in row 64) = sum_ki v_aug[ki,dd] P^T[ki,qi]   (PE, M=65)
  out[qi,dd] = transpose(outT) * (1/l)          (PE transpose + DVE)
Masked ki slots contribute exactly 0 to both out and l because their v_aug row is 0.

The host may compact K/V positions to the unmasked subset (mask is an input, so
this is a pure input-dependent data layout choice); TK is the padded compacted
length. With TK == T no compaction happens and valid() is the raw mask.
"""

import sys

sys.path.insert(0, "/opt/trn_rl_repo")

from contextlib import ExitStack

import numpy as np
import ml_dtypes

import concourse.bass as bass  # noqa: F401
import concourse.tile as tile
from concourse import bacc, mybir
from concourse.bass_utils import run_bass_kernel_spmd
from concourse.masks import make_identity

B, T, C, H, D = 8, 2048, 1024, 16, 64
NCORES = 8
BF16 = mybir.dt.bfloat16
F32 = mybir.dt.float32

# Compaction: K/V positions are gathered to the mask==1 subset on the host and
# padded to TK. TK must be a multiple of 128. TK=T disables compaction.
COMPACT = True
TK_COMPACT = 1152

_nc_cache = {}


def build_nc(TK):
    KT = TK // 128
    nc = bacc.Bacc(None)

    xt_d = nc.dram_tensor("xt", [8, 128, T], BF16, kind="ExternalInput")
    sep_xtc = TK != T
    if sep_xtc:
        xtc_d = nc.dram_tensor("xtc", [8, 128, TK], BF16, kind="ExternalInput")
    wqk_d = nc.dram_tensor("wqk", [8, 128, 2048], BF16, kind="ExternalInput")
    wv_d = nc.dram_tensor("wv", [8, 128, 1024], BF16, kind="ExternalInput")
    bqk_d = nc.dram_tensor("bqk", [128, 16], F32, kind="ExternalInput")
    bv_d = nc.dram_tensor("bv", [1, 1024], BF16, kind="ExternalInput")
    mv_d = nc.dram_tensor("mv", [128, KT], F32, kind="ExternalInput")
    out_d = nc.dram_tensor("out", [H * T, D], F32, kind="ExternalOutput")

    with tile.TileContext(nc) as tc, ExitStack() as ctx:
        const = ctx.enter_context(tc.tile_pool(name="const", bufs=1))

        if sep_xtc:
            xtc = const.tile([128, 8, TK], BF16)
            for i in range(8):
                nc.sync.dma_start(xtc[:, i, :], xtc_d[i])
        wv = const.tile([128, 8, 1024], BF16)
        for i in range(8):
            nc.sync.dma_start(wv[:, i, :], wv_d[i])
        bv = const.tile([1, 1024], BF16)
        nc.sync.dma_start(bv[:], bv_d[:])
        mv = const.tile([128, KT], F32)
        nc.sync.dma_start(mv[:], mv_d[:])
        xt = const.tile([128, 8, T], BF16)
        for i in range(8):
            nc.sync.dma_start(xt[:, i, :], xt_d[i])
        if not sep_xtc:
            xtc = xt
        wqk = const.tile([128, 8, 2048], BF16)
        for i in range(8):
            nc.sync.dma_start(wqk[:, i, :], wqk_d[i])
        bqk = const.tile([128, 16], F32)
        nc.sync.dma_start(bqk[:], bqk_d[:])

        ones_r = const.tile([1, TK], BF16)
        nc.vector.memset(ones_r[:], 1.0)
        ident = const.tile([128, 128], F32)
        make_identity(nc, ident[:])

        qT = const.tile([128, 8, T], BF16)
        kT = const.tile([128, 8, TK], BF16)
        vsb = const.tile([128, KT, 16, 65], BF16)

        psum = ctx.enter_context(tc.tile_pool(name="psum", bufs=1, space="PSUM"))
        sb = ctx.enter_context(tc.tile_pool(name="sb", bufs=1))

        # ---- v projection first (its inputs xtc/wv arrive earliest) ----
        # v[t, c'] = sum_c xT[c, t] Wv[c, c'] + bv  (bias via K=1 matmul);
        # interleaved into vsb[t, :, h, 0:64]; column 64 = valid(t); rows scaled by valid(t)
        nc.vector.memset(vsb[:, :, :, 64:65], 1.0)
        for ti in range(KT):
            for nn in range(2):
                ps = psum.tile([128, 3, 512], F32, tag="s", bufs=2)
                for cc in range(8):
                    nc.tensor.matmul(
                        ps[:, 0, :],
                        xtc[:, cc, ti * 128 : (ti + 1) * 128],
                        wv[:, cc, nn * 512 : (nn + 1) * 512],
                        start=(cc == 0),
                        stop=False,
                    )
                nc.tensor.matmul(
                    ps[:, 0, :],
                    ones_r[:, ti * 128 : (ti + 1) * 128],
                    bv[:, nn * 512 : (nn + 1) * 512],
                    start=False,
                    stop=True,
                )
                nc.vector.tensor_scalar_mul(
                    out=vsb[:, ti, nn * 8 : (nn + 1) * 8, 0:64],
                    in0=ps[:, 0, :].rearrange("p (h d) -> p h d", h=8),
                    scalar1=mv[:, ti : ti + 1],
                )
            nc.vector.tensor_scalar_mul(
                out=vsb[:, ti, :, 64:65],
                in0=vsb[:, ti, :, 64:65],
                scalar1=mv[:, ti : ti + 1],
            )

        # ---- per j: q(j), k(j) projections, then attention for heads 2j, 2j+1 ----
        # Interleaving keeps the PE fed with projection work while ACT runs exp,
        # and lets attention start long before all projections finish.
        GRP = 3
        n_grp = (KT + GRP - 1) // GRP

        def finish_pair(po0, po1, h0, h1, qi):
            # out = transpose(po) scaled by 1/l (l = row 64 of po).
            # Copy both accumulators to SBUF first so their PSUM slots free up
            # before the transposes start cycling through the same tag.
            ots = []
            for po in (po0, po1):
                ot = sb.tile([65, 512], F32, tag="ot", bufs=3)
                nc.vector.tensor_copy(ot[:], po[:])
                nc.vector.reciprocal(ot[64:65, :], ot[64:65, :])
                ots.append(ot)
            for ot, h in zip(ots, (h0, h1)):
                for sub in range(4):
                    pt2 = psum.tile([128, 65], F32, tag="o", bufs=2)
                    nc.tensor.transpose(
                        pt2[:], ot[:, sub * 128 : (sub + 1) * 128], ident[:65, :65]
                    )
                    of = sb.tile([128, 64], F32, tag="of", bufs=4)
                    nc.vector.tensor_scalar_mul(
                        out=of[:], in0=pt2[:, 0:64], scalar1=pt2[:, 64:65]
                    )
                    nc.sync.dma_start(
                        out_d[h * T + qi * 512 + sub * 128 : h * T + qi * 512 + (sub + 1) * 128, :],
                        of[:],
                    )

        for j in range(8):
            # q projection for chunk j
            for tt in range(T // 512):
                ps = psum.tile([128, 3, 512], F32, tag="s", bufs=2)
                for cc in range(8):
                    nc.tensor.matmul(
                        ps[:, 0, :],
                        wqk[:, cc, j * 128 : (j + 1) * 128],
                        xt[:, cc, tt * 512 : (tt + 1) * 512],
                        start=(cc == 0),
                        stop=(cc == 7),
                    )
                nc.vector.tensor_scalar_add(
                    out=qT[:, j, tt * 512 : (tt + 1) * 512],
                    in0=ps[:, 0, :],
                    scalar1=bqk[:, j : j + 1],
                )
            # k projection for chunk j
            for t0 in range(0, TK, 512):
                w = min(512, TK - t0)
                ps = psum.tile([128, 3, 512], F32, tag="s", bufs=2)
                for cc in range(8):
                    nc.tensor.matmul(
                        ps[:, 0, :w],
                        wqk[:, cc, 1024 + j * 128 : 1024 + (j + 1) * 128],
                        xtc[:, cc, t0 : t0 + w],
                        start=(cc == 0),
                        stop=(cc == 7),
                    )
                nc.vector.tensor_scalar_add(
                    out=kT[:, j, t0 : t0 + w],
                    in0=ps[:, 0, :w],
                    scalar1=bqk[:, 8 + j : 9 + j],
                )
            # attention for the head pair (2j, 2j+1); even/odd S-matmuls are
            # emitted adjacently so they run on distinct PE row groups (0-63 /
            # 64-127) concurrently.
            h0, h1 = 2 * j, 2 * j + 1
            for qi in range(4):
                po0 = psum.tile([65, 512], F32, tag="o", bufs=2)
                po1 = psum.tile([65, 512], F32, tag="o", bufs=2)
                for g in range(n_grp):
                    kts = range(g * GRP, min((g + 1) * GRP, KT))
                    gl = len(kts)
                    ps0 = psum.tile([128, 3, 512], F32, tag="s", bufs=2)
                    ps1 = psum.tile([128, 3, 512], F32, tag="s", bufs=2)
                    for idx, kt in enumerate(kts):
                        nc.tensor.matmul(
                            ps0[:, idx, :],
                            kT[0:64, j, kt * 128 : (kt + 1) * 128],
                            qT[0:64, j, qi * 512 : (qi + 1) * 512],
                            start=True,
                            stop=True,
                        )
                        nc.tensor.matmul(
                            ps1[:, idx, :],
                            kT[64:128, j, kt * 128 : (kt + 1) * 128],
                            qT[64:128, j, qi * 512 : (qi + 1) * 512],
                            start=True,
                            stop=True,
                        )
                    pt0 = sb.tile([128, 3, 512], BF16, tag="pt", bufs=4)
                    pt1 = sb.tile([128, 3, 512], BF16, tag="pt", bufs=4)
                    nc.scalar.activation(
                        out=pt0[:, :gl, :],
                        in_=ps0[:, :gl, :],
                        func=mybir.ActivationFunctionType.Exp,
                        scale=0.125,
                    )
                    nc.scalar.activation(
                        out=pt1[:, :gl, :],
                        in_=ps1[:, :gl, :],
                        func=mybir.ActivationFunctionType.Exp,
                        scale=0.125,
                    )
                    for idx, kt in enumerate(kts):
                        nc.tensor.matmul(
                            po0[:],
                            vsb[:, kt, h0, :],
                            pt0[:, idx, :],
                            start=(kt == 0),
                            stop=(kt == KT - 1),
                        )
                        nc.tensor.matmul(
                            po1[:],
                            vsb[:, kt, h1, :],
                            pt1[:, idx, :],
                            start=(kt == 0),
                            stop=(kt == KT - 1),
                        )
                finish_pair(po0, po1, h0, h1, qi)

    nc.compile()
    return nc


def _prep_core(xb, maskb, W_bf, Wv_bf, bqk_np, bv_np, TK):
    """Build the per-core input map for batch element xb (T, C), maskb (T,)."""
    xTb = np.ascontiguousarray(xb.T)  # (C, T) f32
    xt = xTb.astype(ml_dtypes.bfloat16).reshape(8, 128, T)
    m = {"xt": xt, "wqk": W_bf, "wv": Wv_bf, "bqk": bqk_np, "bv": bv_np}
    if TK == T:
        mvv = maskb.astype(np.float32).reshape(TK // 128, 128).T.copy()  # (128, KT)
    else:
        sel = np.nonzero(maskb)[0]
        assert len(sel) <= TK, f"compaction overflow: {len(sel)} > {TK}"
        xc = np.zeros((C, TK), np.float32)
        xc[:, : len(sel)] = xTb[:, sel]
        m["xtc"] = xc.astype(ml_dtypes.bfloat16).reshape(8, 128, TK)
        mvv = np.zeros(TK, np.float32)
        mvv[: len(sel)] = 1.0
        mvv = mvv.reshape(TK // 128, 128).T.copy()
    m["mv"] = mvv
    return m


def make_in_maps(x, mask, W_qkv, b_qkv, TK):
    W_bf = np.ascontiguousarray(W_qkv[:, :2048]).astype(ml_dtypes.bfloat16).reshape(8, 128, 2048)
    Wv_bf = np.ascontiguousarray(W_qkv[:, 2048:]).astype(ml_dtypes.bfloat16).reshape(8, 128, 1024)
    bqk_np = b_qkv[:2048].astype(np.float32).reshape(16, 128).T.copy()  # (128, 16)
    bv_np = b_qkv[2048:].astype(np.float32).reshape(1, 1024).astype(ml_dtypes.bfloat16)
    mask2 = np.asarray(mask).reshape(B, T)
    return [
        _prep_core(np.asarray(x[b]), mask2[b], W_bf, Wv_bf, bqk_np, bv_np, TK)
        for b in range(B)
    ]


def kernel(x, mask, W_qkv, b_qkv):
    mask2 = np.asarray(mask).reshape(B, T)
    TK = T
    if COMPACT:
        need = int(max(mask2.sum(axis=1)))
        if need <= TK_COMPACT:
            TK = TK_COMPACT
    if TK not in _nc_cache:
        _nc_cache[TK] = build_nc(TK)
    nc = _nc_cache[TK]
    in_maps = make_in_maps(x, mask, W_qkv, b_qkv, TK)
    res = run_bass_kernel_spmd(nc, in_maps, core_ids=list(range(NCORES)))
    out = np.stack([res.results[c]["out"] for c in range(NCORES)])  # (B, H*T, D)
    return out.reshape(B, T, C)


if __name__ == "__main__":
    rng = np.random.default_rng(0)
    x = rng.standard_normal((B, T, C), dtype=np.float32)
    mask = (rng.integers(0, 2, (B, 1, 1, T))).astype(np.int32)
    W = (rng.standard_normal((C, 3 * C), dtype=np.float32) * C**-0.5).astype(np.float32)
    bq = (rng.standard_normal(3 * C, dtype=np.float32) * 0.02).astype(np.float32)
    out = kernel(x, mask, W, bq)
    print("out", out.shape, out.dtype)
